# revision 1
# baseline (speedup 1.0000x reference)
"""Deformable conv (DCNv2) kernel for 8 Trainium2 NeuronCores.

Sharding: data-parallel over (batch, H-half): core d handles batch d//2,
output rows [64*(d%2), 64*(d%2)+64). Weights replicated. No cross-device
communication; outputs are concatenated on the host.

Hardcoded problem dims (from the problem spec):
  B=4, CIN=128, H=W=128, COUT=128, K=3, STRIDE=1, PAD=1, DIL=1, GROUPS=1, OG=2
"""

import numpy as np
import jax
import jax.numpy as jnp
from jax import lax
from functools import partial

B, CIN, H, W = 4, 128, 128, 128
COUT, K, PAD, OG = 128, 3, 1, 2
K2 = K * K
HO = 64  # rows per shard (H/2)
NDEV = 8


def _shard_fn(x_b, w_off, weight, row0):
    """Compute output rows [row0, row0+64) for one batch image.

    x_b: [CIN, H, W]; w_off: [54, CIN, 3, 3]; weight: [COUT, CIN, 3, 3];
    row0: int32 scalar.
    """
    C = CIN
    # ---- offset conv on the needed slab (rows row0-1 .. row0+64 incl. pad) ----
    xp = jnp.pad(x_b, ((0, 0), (1, 1), (1, 1)))  # zero pad -> [C, H+2, W+2]
    slab = lax.dynamic_slice(xp, (0, row0, 0), (C, HO + 2, W + 2))
    off_out = lax.conv_general_dilated(
        slab[None], w_off, (1, 1), "VALID",
        dimension_numbers=("NCHW", "OIHW", "NCHW"),
    )[0]  # [54, 64, 128]
    oh, ow, m = jnp.split(off_out, 3, axis=0)  # [18,64,128] each
    mask = jax.nn.sigmoid(m)

    # torchvision-style layout per reference: off[og, k2, (dy,dx)]
    offs = jnp.concatenate([oh, ow], axis=0).reshape(OG, K2, 2, HO, W)

    kg = jnp.arange(K, dtype=x_b.dtype)
    kyf = jnp.repeat(kg, K)  # [K2]
    kxf = jnp.tile(kg, K)    # [K2]
    base_y = (row0 + jnp.arange(HO)).astype(x_b.dtype) - PAD
    base_x = jnp.arange(W, dtype=x_b.dtype) - PAD

    sy = base_y[None, None, :, None] + kyf[None, :, None, None] + offs[:, :, 0]
    sx = base_x[None, None, None, :] + kxf[None, :, None, None] + offs[:, :, 1]
    y0f = jnp.floor(sy)
    x0f = jnp.floor(sx)
    ly = sy - y0f
    lx = sx - x0f
    y0 = y0f.astype(jnp.int32)
    x0 = x0f.astype(jnp.int32)

    Cg = C // OG
    xg = x_b.reshape(OG, Cg, H * W)

    def gather(iy, ix):
        valid = ((iy >= 0) & (iy < H) & (ix >= 0) & (ix < W)).astype(x_b.dtype)
        idx = (jnp.clip(iy, 0, H - 1) * W + jnp.clip(ix, 0, W - 1)).reshape(OG, 1, -1)
        v = jnp.take_along_axis(xg, idx, axis=2).reshape(OG, Cg, K2, HO, W)
        return v * valid[:, None]

    v00 = gather(y0, x0)
    v01 = gather(y0, x0 + 1)
    v10 = gather(y0 + 1, x0)
    v11 = gather(y0 + 1, x0 + 1)
    wy = ly[:, None]
    wx = lx[:, None]
    val = (v00 * (1 - wy) * (1 - wx) + v01 * (1 - wy) * wx
           + v10 * wy * (1 - wx) + v11 * wy * wx)
    val = val * mask.reshape(OG, 1, K2, HO, W)

    cols = val.reshape(C * K2, HO * W)
    wg = weight.reshape(COUT, C * K2)
    out = (wg @ cols).reshape(COUT, HO, W)
    return out


def kernel(x, w_off, b_off, weight, bias):
    x = np.asarray(x, dtype=np.float32)
    w_off = np.asarray(w_off, dtype=np.float32)
    b_off = np.asarray(b_off, dtype=np.float32)
    weight = np.asarray(weight, dtype=np.float32)
    bias = np.asarray(bias, dtype=np.float32)

    devs = jax.devices()[:NDEV]

    # Fold the (constant) offset-conv bias into nothing: b_off is zeros in the
    # spec, but handle it exactly anyway by adding it to the conv weights' output
    # via a post-add inside the shard fn would need plumbing; instead pass it
    # through by adjusting off_out. Simplest exact handling: add via broadcast
    # before use. We incorporate it here by shifting w_off's effective output
    # with a constant channel bias applied in _shard_fn_with_bias below.
    @partial(jax.pmap, devices=devs, static_broadcasted_argnums=())
    def run(x_s, w_off_s, b_off_s, weight_s, bias_s, row0_s):
        out = _shard_fn_bias(x_s, w_off_s, b_off_s, weight_s, row0_s)
        return out + bias_s[:, None, None]

    xs = np.stack([x[d // 2] for d in range(NDEV)])           # [8, C, H, W]
    row0s = np.array([64 * (d % 2) for d in range(NDEV)], dtype=np.int32)
    rep = lambda a: np.broadcast_to(a, (NDEV,) + a.shape).copy()

    outs = run(xs, rep(w_off), rep(b_off), rep(weight), rep(bias), row0s)
    outs = np.asarray(outs)  # [8, COUT, 64, W]
    full = np.empty((B, COUT, H, W), dtype=np.float32)
    for d in range(NDEV):
        full[d // 2, :, 64 * (d % 2):64 * (d % 2) + 64, :] = outs[d]
    return full


def _shard_fn_bias(x_b, w_off, b_off, weight, row0):
    """_shard_fn with the offset-conv bias applied exactly."""
    C = CIN
    xp = jnp.pad(x_b, ((0, 0), (1, 1), (1, 1)))
    slab = lax.dynamic_slice(xp, (0, row0, 0), (C, HO + 2, W + 2))
    off_out = lax.conv_general_dilated(
        slab[None], w_off, (1, 1), "VALID",
        dimension_numbers=("NCHW", "OIHW", "NCHW"),
    )[0] + b_off[:, None, None]
    oh, ow, m = jnp.split(off_out, 3, axis=0)
    mask = jax.nn.sigmoid(m)
    offs = jnp.concatenate([oh, ow], axis=0).reshape(OG, K2, 2, HO, W)

    kg = jnp.arange(K, dtype=x_b.dtype)
    kyf = jnp.repeat(kg, K)
    kxf = jnp.tile(kg, K)
    base_y = (row0 + jnp.arange(HO)).astype(x_b.dtype) - PAD
    base_x = jnp.arange(W, dtype=x_b.dtype) - PAD

    sy = base_y[None, None, :, None] + kyf[None, :, None, None] + offs[:, :, 0]
    sx = base_x[None, None, None, :] + kxf[None, :, None, None] + offs[:, :, 1]
    y0f = jnp.floor(sy)
    x0f = jnp.floor(sx)
    ly = sy - y0f
    lx = sx - x0f
    y0 = y0f.astype(jnp.int32)
    x0 = x0f.astype(jnp.int32)

    Cg = C // OG
    xg = x_b.reshape(OG, Cg, H * W)

    def gather(iy, ix):
        valid = ((iy >= 0) & (iy < H) & (ix >= 0) & (ix < W)).astype(x_b.dtype)
        idx = (jnp.clip(iy, 0, H - 1) * W + jnp.clip(ix, 0, W - 1)).reshape(OG, 1, -1)
        v = jnp.take_along_axis(xg, idx, axis=2).reshape(OG, Cg, K2, HO, W)
        return v * valid[:, None]

    v00 = gather(y0, x0)
    v01 = gather(y0, x0 + 1)
    v10 = gather(y0 + 1, x0)
    v11 = gather(y0 + 1, x0 + 1)
    wy = ly[:, None]
    wx = lx[:, None]
    val = (v00 * (1 - wy) * (1 - wx) + v01 * (1 - wy) * wx
           + v10 * wy * (1 - wx) + v11 * wy * wx)
    val = val * mask.reshape(OG, 1, K2, HO, W)

    cols = val.reshape(C * K2, HO * W)
    wg = weight.reshape(COUT, C * K2)
    out = (wg @ cols).reshape(COUT, HO, W)
    return out
